# revision 28
# baseline (speedup 1.0000x reference)
"""Point-Transformer attention block on 8 Trainium2 NeuronCores.

Shards the points axis N across 8 cores (all ops are pointwise in N).
Per core: channels on SBUF partitions, pixels (k-major: k*TP+n) on the
free dim.  All matmuls run in bf16 (1 cycle/row vs 4 for fp32); inputs
are converted + laid out k-major on the host so every DMA is contiguous.

Fusions vs the reference:
  - x2/x3 in one matmul: stationary [w3; -w2] -> psum [x3_pre+ptf | d]
  - ptf accumulated into the same psum via [pw2; pw2]
  - pw1 block-diag stacked: two 512-px chunks per pass on 128 partitions
  - x1 / xfs / cw1 folded: h = relu(sum_k cw1_k^T d_k + (cw1s@w1)@x
                                    + cw1s@(b1-b2))
  - softmax: exp -> group-sum matmul -> reciprocal; denominator applied
    after the weighted k-sum
  - identity skip-connection via GpSimd add (no matmul)
"""

import numpy as np
import ml_dtypes

B, CIN, N, K = 4, 64, 16384, 16
MID, OUT, PT, SHARE = 64, 128, 8, 8
G = MID // SHARE          # 8 softmax groups
NCORES = 8
NS = N // NCORES          # points per core per batch (2048)
TP = 256                  # points per tile
TPK = TP * K              # pixels per tile (4096)
NT = NS // TP             # tiles per batch (8)
CH = 1024                 # pixel-chunk columns (2 psum banks)
NCH = TPK // CH           # 4 chunks per tile
BF16 = ml_dtypes.bfloat16


def _build_consts(w0, b0, w1, b1, w2, b2, w3, b3, pw1, pw2, cw1, cw2, cb2,
                  wout, bout):
    f32, bf = np.float32, BF16
    c = {}
    w0t = w0.T.astype(np.float32)                                   # [64,128]
    c["w0t2"] = np.ascontiguousarray(
        np.concatenate([w0t, w0t], axis=0), bf)                     # [128,128]
    c["w23t"] = np.ascontiguousarray(
        np.concatenate([w3, -w2], axis=0).T, bf)                    # [128,128]
    pw2t2 = np.concatenate([pw2, pw2], axis=0).T                    # [64,128]
    c["pw2d"] = np.ascontiguousarray(
        np.concatenate([pw2t2, pw2t2], axis=0), bf)                 # [128,128]
    pw1t = pw1.T.astype(np.float32)                                 # [8,64]
    pw1q = np.zeros((OUT, OUT), np.float32)
    for q in range(4):
        for s in range(2):
            pw1q[32 * q + 8 * s:32 * q + 8 * s + PT,
                 s * MID:s * MID + MID] = pw1t
    c["pw1q"] = np.ascontiguousarray(pw1q, bf)                      # [128,128]
    cw1r = cw1.reshape(G, MID, K)
    cw1s = cw1r.sum(-1)                                             # [8,64]
    c["cat"] = np.ascontiguousarray((cw1s @ w1).T, bf)              # [128,8]
    gktp = np.zeros((OUT, (K // 2) * G), np.float32)
    for kp in range(K // 2):
        gktp[0:MID, kp * G:(kp + 1) * G] = cw1r[:, :, 2 * kp].T
        gktp[MID:OUT, kp * G:(kp + 1) * G] = cw1r[:, :, 2 * kp + 1].T
    c["gktp"] = np.ascontiguousarray(gktp, bf)                      # [128,64]
    c["hb"] = np.ascontiguousarray((cw1s @ (b1 - b2))[:, None], f32)  # [8,1]
    c["cw2t"] = np.ascontiguousarray(cw2.T, bf)                     # [8,128]
    c["cb2"] = np.ascontiguousarray(cb2[:, None], f32)              # [128,1]
    bsum = np.zeros((OUT, G), np.float32)
    for g in range(G):
        bsum[g * K:(g + 1) * K, g] = 1.0
    c["bsum"] = np.ascontiguousarray(bsum, bf)                      # [128,8]
    c["grep"] = np.ascontiguousarray(bsum.T, bf)                    # [8,128]
    bksel2 = np.zeros((OUT, (K // 2) * OUT), np.float32)
    for kp in range(K // 2):
        for j in range(2):
            for m in range(MID):
                bksel2[(m % G) * K + 2 * kp + j,
                       kp * OUT + j * MID + m] = 1.0
    c["bksel2"] = np.ascontiguousarray(bksel2, bf)                  # [128,1024]
    fold2 = np.zeros((OUT, MID), np.float32)
    for j in range(2):
        for m in range(MID):
            fold2[j * MID + m, m] = 1.0
    c["fold2"] = np.ascontiguousarray(fold2, bf)                    # [128,64]
    c["woutt"] = np.ascontiguousarray(wout.T, bf)                   # [64,128]
    c["b0"] = np.ascontiguousarray(b0[:, None], f32)                # [128,1]
    b3z = np.zeros((OUT, 1), np.float32)
    b3z[0:MID, 0] = b3
    c["b3z"] = b3z                                                  # [128,1]
    c["bout"] = np.ascontiguousarray(bout[:, None], f32)            # [128,1]
    return c


CONST_SHAPES = dict(
    w0t2=[OUT, OUT], w23t=[OUT, OUT], pw2d=[OUT, OUT], pw1q=[OUT, OUT],
    cat=[OUT, G], gktp=[OUT, K * G // 2], hb=[G, 1], cw2t=[G, OUT],
    cb2=[OUT, 1], bsum=[OUT, G], grep=[G, OUT],
    bksel2=[OUT, K * OUT // 2], fold2=[OUT, MID], woutt=[MID, OUT],
    b0=[OUT, 1], b3z=[OUT, 1], bout=[OUT, 1],
)
CONST_F32 = {"hb", "cb2", "b0", "b3z", "bout"}


def _build_program():
    import concourse.bass as bass
    import concourse.tile as tile
    from concourse import mybir
    from contextlib import ExitStack

    f32 = mybir.dt.float32
    bf16 = mybir.dt.bfloat16
    AF = mybir.ActivationFunctionType
    ALU = mybir.AluOpType

    nc = bass.Bass()
    feats_d = nc.declare_dram_parameter("feats", [B, 2 * CIN, NS * K // 2],
                                        bf16, isOutput=False)
    ppfs_d = nc.declare_dram_parameter("ppfs", [B, OUT, NT * 512], bf16,
                                       isOutput=False)
    cdram = {k: nc.declare_dram_parameter(
                 k, v, f32 if k in CONST_F32 else bf16, isOutput=False)
             for k, v in CONST_SHAPES.items()}
    out_d = nc.declare_dram_parameter("out", [B, OUT, NS], bf16, isOutput=True)

    with tile.TileContext(nc) as tc, ExitStack() as ctx:
        consts = ctx.enter_context(tc.tile_pool(name="consts", bufs=1))
        ct = {k: consts.tile_from(v[:], name=k) for k, v in cdram.items()}

        io = ctx.enter_context(tc.tile_pool(name="io", bufs=3))
        sb4 = ctx.enter_context(tc.tile_pool(name="sb4", bufs=4))
        sb3 = ctx.enter_context(tc.tile_pool(name="sb3", bufs=3))
        sb = ctx.enter_context(tc.tile_pool(name="sb", bufs=2))
        pt_pool = ctx.enter_context(tc.tile_pool(name="pt", bufs=3))
        ps_pix = ctx.enter_context(tc.tile_pool(name="ps_pix", bufs=3,
                                                space="PSUM"))
        ps_pt = ctx.enter_context(tc.tile_pool(name="ps_pt", bufs=2,
                                               space="PSUM"))

        def emit_s1(b, t):
            """Pixel phase: xn / r / [x3|d] + parity repacks (DMA)."""
            pk = slice(t * TPK // 2, (t + 1) * TPK // 2)
            ph = slice(t * 512, (t + 1) * 512)

            ft = io.tile([OUT, TPK // 2], bf16, tag="ft")
            nc.sync.dma_start(ft[:], feats_d[b, :, pk])
            pf = io.tile([OUT, 512], bf16, tag="pf")
            nc.sync.dma_start(pf[:], ppfs_d[b, :, ph])

            xn = sb4.tile([OUT, TPK], bf16, tag="xn")
            x3dd = sb.tile([OUT, TPK], bf16, tag="x3dd")
            r = sb.tile([OUT, TPK // 2], bf16, tag="r")

            # r: 4-way row-tiled pw1 (concurrent in PE quadrant rows)
            for i in range(2):
                p = ps_pix.tile([OUT, CH], f32, tag="pix")
                for j in range(2):
                    q = 2 * i + j
                    nc.tensor.matmul(out=p[:, j * 512:(j + 1) * 512],
                                     lhsT=ct["pw1q"][32 * q:32 * q + 16, :],
                                     rhs=pf[32 * q:32 * q + 16, :],
                                     start=True, stop=True,
                                     tile_position=(32 * q, 0))
                nc.vector.tensor_scalar_max(
                    r[:, i * CH:(i + 1) * CH], p[:], 0.0)

            # xn = relu(w0 @ feats + b0): k-halves row-tiled concurrent
            for c in range(NCH):
                p = ps_pix.tile([OUT, CH], f32, tag="pix")
                cs = slice(c * 512, (c + 1) * 512)
                nc.tensor.matmul(out=p[:, 0:512],
                                 lhsT=ct["w0t2"][0:CIN, :], rhs=ft[0:CIN, cs],
                                 start=True, stop=True)
                nc.tensor.matmul(out=p[:, 512:1024],
                                 lhsT=ct["w0t2"][CIN:OUT, :],
                                 rhs=ft[CIN:OUT, cs], start=True, stop=True)
                xo = xn[:].rearrange("p (h n) -> p h n", h=2)[:, :,
                                                             c * 512:(c + 1) * 512]
                pv = p[:].rearrange("p (h n) -> p h n", h=2)
                nc.scalar.activation(xo, pv, AF.Relu, bias=ct["b0"][:])

            # psum = [w3@xn + pw2@r | -w2@xn + pw2@r] -> x3 (+b3), d
            for i in range(NCH):
                p = ps_pix.tile([OUT, CH], f32, tag="pix")
                cs = slice(i * CH, (i + 1) * CH)
                for j in range(2):
                    js = slice(i * CH + j * 512, i * CH + (j + 1) * 512)
                    nc.tensor.matmul(out=p[:, j * 512:(j + 1) * 512],
                                     lhsT=ct["w23t"][:],
                                     rhs=xn[:, js], start=True, stop=False)
                for j in range(2):
                    nc.tensor.matmul(
                        out=p[:, j * 512:(j + 1) * 512],
                        lhsT=ct["pw2d"][j * MID:(j + 1) * MID, :],
                        rhs=r[j * MID:(j + 1) * MID, i * 512:(i + 1) * 512],
                        start=False, stop=True)
                if i < 2:
                    nc.scalar.activation(x3dd[:, cs], p[:], AF.Identity,
                                         bias=ct["b3z"][:])
                else:
                    nc.vector.tensor_scalar_add(x3dd[:, cs], p[:],
                                                ct["b3z"][:])

            # parity repack via SBUF->SBUF DMA: top=even k, bottom=odd k
            x3p = sb3.tile([OUT, TPK // 2], bf16, tag="x3p")
            ddp = sb3.tile([OUT, TPK // 2], bf16, tag="ddp")
            v = x3dd[:].rearrange("p (k2 two n) -> p two k2 n", two=2, n=TP)
            x3pv = x3p[:].rearrange("p (k2 n) -> p k2 n", n=TP)
            ddpv = ddp[:].rearrange("p (k2 n) -> p k2 n", n=TP)
            nc.sync.dma_start(x3pv[0:MID], v[0:MID, 0])
            nc.sync.dma_start(x3pv[MID:OUT], v[0:MID, 1])
            nc.sync.dma_start(ddpv[0:MID], v[MID:OUT, 0])
            nc.sync.dma_start(ddpv[MID:OUT], v[MID:OUT, 1])
            return dict(b=b, t=t, xn=xn, x3p=x3p, ddp=ddp)

        def emit_s2(st):
            """h = relu(sum_k cw1_k^T d_k + Ca x + hb); paired contraction."""
            sm1 = ps_pt.tile([OUT, 512], f32, tag="pt")
            hps = sm1[0:G, 256:512]
            ddp = st["ddp"]
            for kp in range(K // 2):
                nc.tensor.matmul(
                    out=hps, lhsT=ct["gktp"][:, kp * G:(kp + 1) * G],
                    rhs=ddp[:, kp * TP:(kp + 1) * TP],
                    start=(kp == 0), stop=False)
            nc.tensor.matmul(out=hps, lhsT=ct["cat"][:],
                             rhs=st["xn"][:, 0:TP], start=False, stop=True)
            h = pt_pool.tile([G, TP], bf16, tag="h")
            nc.scalar.activation(h[:], hps, AF.Relu, bias=ct["hb"][:])
            st.update(sm1=sm1, h=h)

        def emit_cw2(st):
            # e = exp(cw2 @ h + cb2)                          [128, TP]
            sm1 = st["sm1"]
            wlps = sm1[:, 0:256]
            nc.tensor.matmul(out=wlps, lhsT=ct["cw2t"][:], rhs=st["h"][:],
                             start=True, stop=True)
            e = pt_pool.tile([OUT, TP], bf16, tag="e")
            nc.scalar.activation(e[:], wlps, AF.Exp, bias=ct["cb2"][:])
            st.update(e=e)

        def emit_s3a(st):
            sm1, x3p, e = st["sm1"], st["x3p"], st["e"]

            # rs = 1 / group-sum(e)   (out base partition 32-aligned)
            sps = sm1[32:32 + G, 256:512]
            nc.tensor.matmul(out=sps, lhsT=ct["bsum"][:], rhs=e[:],
                             start=True, stop=True)
            rs32 = pt_pool.tile([G, TP], f32, tag="rs32")
            nc.vector.reciprocal(rs32[:], sps)
            rs16 = pt_pool.tile([G, TP], bf16, tag="rs16")
            nc.gpsimd.tensor_copy(rs16[:], rs32[:])

            # en = softmax(e): e * (1/groupsum) broadcast     [128, TP]
            rbps = sm1[:, 0:256]  # reuse wl region after e drained
            nc.tensor.matmul(out=rbps, lhsT=ct["grep"][:], rhs=rs16[:],
                             start=True, stop=True)
            en = pt_pool.tile([OUT, TP], bf16, tag="en")
            nc.vector.tensor_mul(en[:], e[:], rbps)

            # wf pairs: psum [128,TP] = [wf_{2kp} | wf_{2kp+1}]; q2 = wf*x3p
            q2 = sb.tile([OUT, TPK // 2], bf16, tag="q2")
            for inst in range(2):
                wfp = ps_pix.tile([OUT, CH], f32, tag="pix")
                for j in range(NCH):
                    kp = inst * NCH + j
                    nc.tensor.matmul(
                        out=wfp[:, j * TP:(j + 1) * TP],
                        lhsT=ct["bksel2"][:, kp * OUT:(kp + 1) * OUT],
                        rhs=en[:], start=True, stop=True)
                cs = slice(inst * CH, (inst + 1) * CH)
                nc.vector.tensor_mul(q2[:, cs], wfp[:], x3p[:, cs])

            # U = sum_k q: pairwise-add tree on GpSimd (SBUF bf16)
            t1 = pt_pool.tile([OUT, CH], bf16, tag="t1")
            t2 = pt_pool.tile([OUT, 512], bf16, tag="t2")
            ueo = pt_pool.tile([OUT, TP], bf16, tag="ueo")
            with nc.allow_low_precision("8-term bf16 partial sums"):
                nc.gpsimd.tensor_add(t1[:], q2[:, 0:CH], q2[:, CH:2 * CH])
                nc.gpsimd.tensor_add(t2[:], t1[:, 0:512], t1[:, 512:CH])
                nc.gpsimd.tensor_add(ueo[:], t2[:, 0:TP], t2[:, TP:512])
            st.update(ueo=ueo)

        def emit_fold(st):
            # fold parity halves of ueo via matmul; o = relu(U)
            wop = ps_pix.tile([OUT, CH], f32, tag="pix")
            ups = wop[0:MID, 256:512]
            nc.tensor.matmul(out=ups, lhsT=ct["fold2"][:], rhs=st["ueo"][:],
                             start=True, stop=True)
            o = pt_pool.tile([MID, TP], bf16, tag="o")
            nc.scalar.activation(o[:], ups, AF.Relu)
            st.update(o=o, wop=wop)

        def emit_s3b(st):
            b, t = st["b"], st["t"]
            # out = wout @ o + bout + x                       [128, TP]
            ops_ = st["wop"][:, 0:256]
            nc.tensor.matmul(out=ops_, lhsT=ct["woutt"][:], rhs=st["o"][:],
                             start=True, stop=True)
            res = pt_pool.tile([OUT, TP], bf16, tag="res")
            nc.vector.tensor_scalar_add(res[:], ops_, ct["bout"][:])
            outt = pt_pool.tile([OUT, TP], bf16, tag="outt")
            nc.gpsimd.tensor_add(outt[:], res[:], st["xn"][:, 0:TP])
            nc.sync.dma_start(out_d[b, :, t * TP:(t + 1) * TP], outt[:])

        tiles = [(b, t) for b in range(B) for t in range(NT)] + [None] * 3
        p1 = p2 = p3 = None
        for tile_idx in tiles:
            if p3 is not None:
                emit_fold(p3)
            if p1 is not None:
                emit_s2(p1)
            if p2 is not None:
                emit_s3a(p2)
            if p1 is not None:
                emit_cw2(p1)
            cur = emit_s1(*tile_idx) if tile_idx is not None else None
            if p3 is not None:
                emit_s3b(p3)
            p3, p2, p1 = p2, p1, cur

    return nc


def _legalize_waits(nc):
    """This toolchain's walrus rejects >1 sync-wait per instruction; hoist
    extra waits onto same-engine event-semaphore instructions just before."""
    from concourse import mybir

    n_split = 0
    for fn in nc.m.functions:
        for bb in fn.blocks:
            insts = bb.instructions
            new_list = []
            for inst in insts:
                si = inst.sync_info
                if si is not None and si.on_wait is not None and len(si.on_wait) > 1:
                    waits = list(si.on_wait)
                    for j, w in enumerate(waits[:-1]):
                        ev = mybir.InstEventSemaphore(
                            name=f"{inst.name}-lw{j}", ins=[], outs=[])
                        ev.engine = inst.engine
                        ev.sync_info = mybir.SyncInfo(on_wait=[w], on_update=[])
                        new_list.append(ev)
                        n_split += 1
                    inst.sync_info = mybir.SyncInfo(
                        on_wait=[waits[-1]], on_update=list(si.on_update))
                new_list.append(inst)
            if len(new_list) != len(insts):
                insts[:] = new_list
    return n_split


def _pack_feats(x):
    # [B, CIN, NS, K] -> per-tile k-major cols, k-halves stacked on
    # partitions: out[b, h*64+c, t, k8*TP+n] = x[b, c, t*TP+n, h*8+k8]
    a = x.reshape(B, CIN, NT, TP, 2, K // 2).transpose(0, 4, 1, 2, 5, 3)
    return np.ascontiguousarray(a, BF16).reshape(B, 2 * CIN, NS * K // 2)


def _pack_ppfs(x):
    # [B, PT, NS, K] -> per-tile [128, 512] for 4-way row-tiled pw1:
    # partition 32q+8s+c holds pixel cols q*1024+s*512+j (j = k1*TP+n)
    a = x.reshape(B, PT, NT, TP, K).transpose(0, 1, 2, 4, 3)  # [B,8,NT,K,TP]
    out = np.zeros((B, OUT, NT, 512), BF16)
    for k in range(K):
        q, s, k1 = k // 4, (k // 2) % 2, k % 2
        out[:, 32 * q + 8 * s:32 * q + 8 * s + PT, :,
            k1 * TP:(k1 + 1) * TP] = a[:, :, :, k, :]
    return np.ascontiguousarray(out).reshape(B, OUT, NT * 512)


LAST_RESULTS = None


def kernel(sm_feats, sm_ppfs, w0, b0, w1, b1, w2, b2, w3, b3,
           pw1, pw2, cw1, cw2, cb2, wout, bout):
    global LAST_RESULTS
    from concourse.bass_utils import run_bass_kernel_spmd

    consts = _build_consts(w0, b0, w1, b1, w2, b2, w3, b3, pw1, pw2,
                           cw1, cw2, cb2, wout, bout)
    nc = _build_program()
    _legalize_waits(nc)

    in_maps = []
    for i in range(NCORES):
        sl = slice(i * NS, (i + 1) * NS)
        m = dict(consts)
        m["feats"] = _pack_feats(np.ascontiguousarray(sm_feats[:, :, sl, :]))
        m["ppfs"] = _pack_ppfs(np.ascontiguousarray(sm_ppfs[:, :, sl, :]))
        in_maps.append(m)

    res = run_bass_kernel_spmd(nc, in_maps, list(range(NCORES)))
    LAST_RESULTS = res
    shards = [res.results[i]["out"].astype(np.float32) for i in range(NCORES)]
    return np.concatenate(shards, axis=2)


# revision 32
# speedup vs baseline: 1.1488x; 1.1488x over previous
"""Point-Transformer attention block on 8 Trainium2 NeuronCores.

Shards the points axis N across 8 cores (all ops are pointwise in N).
Per core: channels on SBUF partitions, pixels (k-major: k*TP+n) on the
free dim.  All matmuls run in bf16 (1 cycle/row vs 4 for fp32); inputs
are converted + laid out k-major on the host so every DMA is contiguous.

Fusions vs the reference:
  - x2/x3 in one matmul: stationary [w3; -w2] -> psum [x3_pre+ptf | d]
  - ptf accumulated into the same psum via [pw2; pw2]
  - pw1 block-diag stacked: two 512-px chunks per pass on 128 partitions
  - x1 / xfs / cw1 folded: h = relu(sum_k cw1_k^T d_k + (cw1s@w1)@x
                                    + cw1s@(b1-b2))
  - softmax: exp -> group-sum matmul -> reciprocal; denominator applied
    after the weighted k-sum
  - identity skip-connection via GpSimd add (no matmul)
"""

import numpy as np
import ml_dtypes

B, CIN, N, K = 4, 64, 16384, 16
MID, OUT, PT, SHARE = 64, 128, 8, 8
G = MID // SHARE          # 8 softmax groups
NCORES = 8
NS = N // NCORES          # points per core per batch (2048)
TP = 256                  # points per tile
TPK = TP * K              # pixels per tile (4096)
NT = NS // TP             # tiles per batch (8)
CH = 1024                 # pixel-chunk columns (2 psum banks)
NCH = TPK // CH           # 4 chunks per tile
BF16 = ml_dtypes.bfloat16


def _build_consts(w0, b0, w1, b1, w2, b2, w3, b3, pw1, pw2, cw1, cw2, cb2,
                  wout, bout):
    f32, bf = np.float32, BF16
    c = {}
    w0t = w0.T.astype(np.float32)                                   # [64,128]
    c["w0t2"] = np.ascontiguousarray(
        np.concatenate([w0t, w0t], axis=0), bf)                     # [128,128]
    c["w23t"] = np.ascontiguousarray(
        np.concatenate([w3, -w2], axis=0).T, bf)                    # [128,128]
    pw2t2 = np.concatenate([pw2, pw2], axis=0).T                    # [64,128]
    c["pw2d"] = np.ascontiguousarray(
        np.concatenate([pw2t2, pw2t2], axis=0), bf)                 # [128,128]
    pw1t = pw1.T.astype(np.float32)                                 # [8,64]
    pw1q = np.zeros((OUT, OUT), np.float32)
    for q in range(4):
        for s in range(2):
            pw1q[32 * q + 8 * s:32 * q + 8 * s + PT,
                 s * MID:s * MID + MID] = pw1t
    c["pw1q"] = np.ascontiguousarray(pw1q, bf)                      # [128,128]
    cw1r = cw1.reshape(G, MID, K)
    cw1s = cw1r.sum(-1)                                             # [8,64]
    c["cat"] = np.ascontiguousarray((cw1s @ w1).T, bf)              # [128,8]
    gktp = np.zeros((OUT, (K // 2) * G), np.float32)
    for kp in range(K // 2):
        gktp[0:MID, kp * G:(kp + 1) * G] = cw1r[:, :, 2 * kp].T
        gktp[MID:OUT, kp * G:(kp + 1) * G] = cw1r[:, :, 2 * kp + 1].T
    c["gktp"] = np.ascontiguousarray(gktp, bf)                      # [128,64]
    c["hb"] = np.ascontiguousarray((cw1s @ (b1 - b2))[:, None], f32)  # [8,1]
    c["cw2t"] = np.ascontiguousarray(cw2.T, bf)                     # [8,128]
    c["cb2"] = np.ascontiguousarray(cb2[:, None], f32)              # [128,1]
    bsum = np.zeros((OUT, G), np.float32)
    for g in range(G):
        bsum[g * K:(g + 1) * K, g] = 1.0
    c["bsum"] = np.ascontiguousarray(bsum, bf)                      # [128,8]
    c["grep"] = np.ascontiguousarray(bsum.T, bf)                    # [8,128]
    bksel2 = np.zeros((OUT, (K // 2) * OUT), np.float32)
    for kp in range(K // 2):
        for j in range(2):
            for m in range(MID):
                bksel2[(m % G) * K + 2 * kp + j,
                       kp * OUT + j * MID + m] = 1.0
    c["bksel2"] = np.ascontiguousarray(bksel2, bf)                  # [128,1024]
    fold2 = np.zeros((OUT, MID), np.float32)
    for j in range(2):
        for m in range(MID):
            fold2[j * MID + m, m] = 1.0
    c["fold2"] = np.ascontiguousarray(fold2, bf)                    # [128,64]
    c["woutt"] = np.ascontiguousarray(wout.T, bf)                   # [64,128]
    c["b0"] = np.ascontiguousarray(b0[:, None], f32)                # [128,1]
    b3z = np.zeros((OUT, 1), np.float32)
    b3z[0:MID, 0] = b3
    c["b3z"] = b3z                                                  # [128,1]
    c["bout"] = np.ascontiguousarray(bout[:, None], f32)            # [128,1]
    return c


CONST_SHAPES = dict(
    w0t2=[OUT, OUT], w23t=[OUT, OUT], pw2d=[OUT, OUT], pw1q=[OUT, OUT],
    cat=[OUT, G], gktp=[OUT, K * G // 2], hb=[G, 1], cw2t=[G, OUT],
    cb2=[OUT, 1], bsum=[OUT, G], grep=[G, OUT],
    bksel2=[OUT, K * OUT // 2], fold2=[OUT, MID], woutt=[MID, OUT],
    b0=[OUT, 1], b3z=[OUT, 1], bout=[OUT, 1],
)
CONST_F32 = {"hb", "cb2", "b0", "b3z", "bout"}


def _build_program():
    import concourse.bass as bass
    import concourse.tile as tile
    from concourse import mybir
    from contextlib import ExitStack

    f32 = mybir.dt.float32
    bf16 = mybir.dt.bfloat16
    AF = mybir.ActivationFunctionType
    ALU = mybir.AluOpType

    nc = bass.Bass()
    feats_d = nc.declare_dram_parameter("feats", [B, 2 * CIN, NS * K // 2],
                                        bf16, isOutput=False)
    ppfs_d = nc.declare_dram_parameter("ppfs", [B, OUT, NT * 512], bf16,
                                       isOutput=False)
    cdram = {k: nc.declare_dram_parameter(
                 k, v, f32 if k in CONST_F32 else bf16, isOutput=False)
             for k, v in CONST_SHAPES.items()}
    out_d = nc.declare_dram_parameter("out", [B, OUT, NS], bf16, isOutput=True)

    with tile.TileContext(nc) as tc, ExitStack() as ctx:
        consts = ctx.enter_context(tc.tile_pool(name="consts", bufs=1))
        ct = {k: consts.tile_from(v[:], name=k) for k, v in cdram.items()}

        io = ctx.enter_context(tc.tile_pool(name="io", bufs=3))
        sb4 = ctx.enter_context(tc.tile_pool(name="sb4", bufs=4))
        sb3 = ctx.enter_context(tc.tile_pool(name="sb3", bufs=3))
        sb = ctx.enter_context(tc.tile_pool(name="sb", bufs=2))
        pt_pool = ctx.enter_context(tc.tile_pool(name="pt", bufs=3))
        ps_pix = ctx.enter_context(tc.tile_pool(name="ps_pix", bufs=2,
                                                space="PSUM"))
        ps_wf = ctx.enter_context(tc.tile_pool(name="ps_wf", bufs=2,
                                               space="PSUM"))
        ps_pt = ctx.enter_context(tc.tile_pool(name="ps_pt", bufs=2,
                                               space="PSUM"))

        def emit_s1(b, t):
            """Pixel phase: xn / r / [x3|d] + parity repacks (DMA)."""
            pk = slice(t * TPK // 2, (t + 1) * TPK // 2)
            ph = slice(t * 512, (t + 1) * 512)

            ft = io.tile([OUT, TPK // 2], bf16, tag="ft")
            nc.sync.dma_start(ft[:], feats_d[b, :, pk])
            pf = io.tile([OUT, 512], bf16, tag="pf")
            nc.sync.dma_start(pf[:], ppfs_d[b, :, ph])

            xn = sb4.tile([OUT, TPK], bf16, tag="xn")
            x3dd = sb.tile([OUT, TPK], bf16, tag="x3dd")
            r = sb.tile([OUT, TPK // 2], bf16, tag="r")

            # r: 4-way row-tiled pw1 (concurrent in PE quadrant rows)
            for i in range(2):
                p = ps_pix.tile([OUT, CH], f32, tag="pix")
                for j in range(2):
                    q = 2 * i + j
                    nc.tensor.matmul(out=p[:, j * 512:(j + 1) * 512],
                                     lhsT=ct["pw1q"][32 * q:32 * q + 16, :],
                                     rhs=pf[32 * q:32 * q + 16, :],
                                     start=True, stop=True,
                                     tile_position=(32 * q, 0))
                nc.vector.tensor_scalar_max(
                    r[:, i * CH:(i + 1) * CH], p[:], 0.0)

            # xn = relu(w0 @ feats + b0): k-halves row-tiled concurrent
            for c in range(NCH):
                p = ps_pix.tile([OUT, CH], f32, tag="pix")
                cs = slice(c * 512, (c + 1) * 512)
                nc.tensor.matmul(out=p[:, 0:512],
                                 lhsT=ct["w0t2"][0:CIN, :], rhs=ft[0:CIN, cs],
                                 start=True, stop=True)
                nc.tensor.matmul(out=p[:, 512:1024],
                                 lhsT=ct["w0t2"][CIN:OUT, :],
                                 rhs=ft[CIN:OUT, cs], start=True, stop=True)
                xo = xn[:].rearrange("p (h n) -> p h n", h=2)[:, :,
                                                             c * 512:(c + 1) * 512]
                pv = p[:].rearrange("p (h n) -> p h n", h=2)
                nc.scalar.activation(xo, pv, AF.Relu, bias=ct["b0"][:])

            # psum = [w3@xn + pw2@r | -w2@xn + pw2@r] -> x3 (+b3), d
            for i in range(NCH):
                p = ps_pix.tile([OUT, CH], f32, tag="pix")
                cs = slice(i * CH, (i + 1) * CH)
                for j in range(2):
                    js = slice(i * CH + j * 512, i * CH + (j + 1) * 512)
                    nc.tensor.matmul(out=p[:, j * 512:(j + 1) * 512],
                                     lhsT=ct["w23t"][:],
                                     rhs=xn[:, js], start=True, stop=False)
                for j in range(2):
                    nc.tensor.matmul(
                        out=p[:, j * 512:(j + 1) * 512],
                        lhsT=ct["pw2d"][j * MID:(j + 1) * MID, :],
                        rhs=r[j * MID:(j + 1) * MID, i * 512:(i + 1) * 512],
                        start=False, stop=True)
                if i < 3:
                    nc.scalar.activation(x3dd[:, cs], p[:], AF.Identity,
                                         bias=ct["b3z"][:])
                else:
                    nc.vector.tensor_scalar_add(x3dd[:, cs], p[:],
                                                ct["b3z"][:])

            # parity repack via SBUF->SBUF DMA: top=even k, bottom=odd k
            x3p = sb3.tile([OUT, TPK // 2], bf16, tag="x3p")
            ddp = sb3.tile([OUT, TPK // 2], bf16, tag="ddp")
            v = x3dd[:].rearrange("p (k2 two n) -> p two k2 n", two=2, n=TP)
            x3pv = x3p[:].rearrange("p (k2 n) -> p k2 n", n=TP)
            ddpv = ddp[:].rearrange("p (k2 n) -> p k2 n", n=TP)
            nc.sync.dma_start(x3pv[0:MID], v[0:MID, 0])
            nc.sync.dma_start(x3pv[MID:OUT], v[0:MID, 1])
            nc.sync.dma_start(ddpv[0:MID], v[MID:OUT, 0])
            nc.sync.dma_start(ddpv[MID:OUT], v[MID:OUT, 1])
            return dict(b=b, t=t, xn=xn, x3p=x3p, ddp=ddp)

        def emit_s2(st):
            """h = relu(sum_k cw1_k^T d_k + Ca x + hb); paired contraction."""
            sm1 = ps_pt.tile([OUT, 512], f32, tag="pt")
            hps = sm1[0:G, 256:512]
            ddp = st["ddp"]
            for kp in range(K // 2):
                nc.tensor.matmul(
                    out=hps, lhsT=ct["gktp"][:, kp * G:(kp + 1) * G],
                    rhs=ddp[:, kp * TP:(kp + 1) * TP],
                    start=(kp == 0), stop=False)
            nc.tensor.matmul(out=hps, lhsT=ct["cat"][:],
                             rhs=st["xn"][:, 0:TP], start=False, stop=True)
            h = pt_pool.tile([G, TP], bf16, tag="h")
            nc.scalar.activation(h[:], hps, AF.Relu, bias=ct["hb"][:])
            st.update(sm1=sm1, h=h)

        def emit_cw2(st):
            # e = exp(cw2 @ h + cb2)                          [128, TP]
            sm1 = st["sm1"]
            wlps = sm1[:, 0:256]
            nc.tensor.matmul(out=wlps, lhsT=ct["cw2t"][:], rhs=st["h"][:],
                             start=True, stop=True)
            e = pt_pool.tile([OUT, TP], bf16, tag="e")
            nc.scalar.activation(e[:], wlps, AF.Exp, bias=ct["cb2"][:])
            st.update(e=e)

        def emit_s3a(st):
            sm1, x3p, e = st["sm1"], st["x3p"], st["e"]

            # rs = 1 / group-sum(e)   (out base partition 32-aligned)
            sps = sm1[32:32 + G, 256:512]
            nc.tensor.matmul(out=sps, lhsT=ct["bsum"][:], rhs=e[:],
                             start=True, stop=True)
            rs32 = pt_pool.tile([G, TP], f32, tag="rs32")
            nc.vector.reciprocal(rs32[:], sps)
            rs16 = pt_pool.tile([G, TP], bf16, tag="rs16")
            nc.gpsimd.tensor_copy(rs16[:], rs32[:])

            # en = softmax(e): e * (1/groupsum) broadcast     [128, TP]
            rbps = sm1[:, 0:256]  # reuse wl region after e drained
            nc.tensor.matmul(out=rbps, lhsT=ct["grep"][:], rhs=rs16[:],
                             start=True, stop=True)
            en = pt_pool.tile([OUT, TP], bf16, tag="en")
            nc.vector.tensor_mul(en[:], e[:], rbps)

            # wf pairs: psum [128,TP] = [wf_{2kp} | wf_{2kp+1}]; q2 = wf*x3p
            q2 = sb.tile([OUT, TPK // 2], bf16, tag="q2")
            for inst in range(4):
                wfp = ps_wf.tile([OUT, 512], f32, tag="wf")
                for j in range(2):
                    kp = inst * 2 + j
                    nc.tensor.matmul(
                        out=wfp[:, j * TP:(j + 1) * TP],
                        lhsT=ct["bksel2"][:, kp * OUT:(kp + 1) * OUT],
                        rhs=en[:], start=True, stop=True)
                cs = slice(inst * 512, (inst + 1) * 512)
                nc.vector.tensor_mul(q2[:, cs], wfp[:], x3p[:, cs])

            # U = sum_k q: pairwise-add tree on GpSimd (SBUF bf16)
            t1 = pt_pool.tile([OUT, CH], bf16, tag="t1")
            t2 = pt_pool.tile([OUT, 512], bf16, tag="t2")
            ueo = pt_pool.tile([OUT, TP], bf16, tag="ueo")
            with nc.allow_low_precision("8-term bf16 partial sums"):
                nc.gpsimd.tensor_add(t1[:], q2[:, 0:CH], q2[:, CH:2 * CH])
                nc.gpsimd.tensor_add(t2[:], t1[:, 0:512], t1[:, 512:CH])
                nc.gpsimd.tensor_add(ueo[:], t2[:, 0:TP], t2[:, TP:512])
            st.update(ueo=ueo)

        def emit_fold(st):
            # fold parity halves of ueo via matmul; o = relu(U)
            wop = ps_wf.tile([OUT, 512], f32, tag="wf")
            ups = wop[0:MID, 256:512]
            nc.tensor.matmul(out=ups, lhsT=ct["fold2"][:], rhs=st["ueo"][:],
                             start=True, stop=True)
            o = pt_pool.tile([MID, TP], bf16, tag="o")
            nc.scalar.activation(o[:], ups, AF.Relu)
            st.update(o=o, wop=wop)

        def emit_s3b(st):
            b, t = st["b"], st["t"]
            # out = wout @ o + bout + x                       [128, TP]
            ops_ = st["wop"][:, 0:256]
            nc.tensor.matmul(out=ops_, lhsT=ct["woutt"][:], rhs=st["o"][:],
                             start=True, stop=True)
            res = pt_pool.tile([OUT, TP], bf16, tag="res")
            nc.vector.tensor_scalar_add(res[:], ops_, ct["bout"][:])
            outt = pt_pool.tile([OUT, TP], bf16, tag="outt")
            nc.gpsimd.tensor_add(outt[:], res[:], st["xn"][:, 0:TP])
            nc.sync.dma_start(out_d[b, :, t * TP:(t + 1) * TP], outt[:])

        tiles = [(b, t) for b in range(B) for t in range(NT)] + [None] * 3
        p1 = p2 = p3 = None
        for tile_idx in tiles:
            if p3 is not None:
                emit_fold(p3)
            if p1 is not None:
                emit_s2(p1)
            if p2 is not None:
                emit_s3a(p2)
            if p1 is not None:
                emit_cw2(p1)
            cur = emit_s1(*tile_idx) if tile_idx is not None else None
            if p3 is not None:
                emit_s3b(p3)
            p3, p2, p1 = p2, p1, cur

    return nc


def _legalize_waits(nc):
    """This toolchain's walrus rejects >1 sync-wait per instruction; hoist
    extra waits onto same-engine event-semaphore instructions just before."""
    from concourse import mybir

    n_split = 0
    for fn in nc.m.functions:
        for bb in fn.blocks:
            insts = bb.instructions
            new_list = []
            for inst in insts:
                si = inst.sync_info
                if si is not None and si.on_wait is not None and len(si.on_wait) > 1:
                    waits = list(si.on_wait)
                    for j, w in enumerate(waits[:-1]):
                        ev = mybir.InstEventSemaphore(
                            name=f"{inst.name}-lw{j}", ins=[], outs=[])
                        ev.engine = inst.engine
                        ev.sync_info = mybir.SyncInfo(on_wait=[w], on_update=[])
                        new_list.append(ev)
                        n_split += 1
                    inst.sync_info = mybir.SyncInfo(
                        on_wait=[waits[-1]], on_update=list(si.on_update))
                new_list.append(inst)
            if len(new_list) != len(insts):
                insts[:] = new_list
    return n_split


def _pack_feats(x):
    # [B, CIN, NS, K] -> per-tile k-major cols, k-halves stacked on
    # partitions: out[b, h*64+c, t, k8*TP+n] = x[b, c, t*TP+n, h*8+k8]
    a = x.reshape(B, CIN, NT, TP, 2, K // 2).transpose(0, 4, 1, 2, 5, 3)
    return np.ascontiguousarray(a, BF16).reshape(B, 2 * CIN, NS * K // 2)


def _pack_ppfs(x):
    # [B, PT, NS, K] -> per-tile [128, 512] for 4-way row-tiled pw1:
    # partition 32q+8s+c holds pixel cols q*1024+s*512+j (j = k1*TP+n)
    a = x.reshape(B, PT, NT, TP, K).transpose(0, 1, 2, 4, 3)  # [B,8,NT,K,TP]
    out = np.zeros((B, OUT, NT, 512), BF16)
    for k in range(K):
        q, s, k1 = k // 4, (k // 2) % 2, k % 2
        out[:, 32 * q + 8 * s:32 * q + 8 * s + PT, :,
            k1 * TP:(k1 + 1) * TP] = a[:, :, :, k, :]
    return np.ascontiguousarray(out).reshape(B, OUT, NT * 512)


LAST_RESULTS = None


def kernel(sm_feats, sm_ppfs, w0, b0, w1, b1, w2, b2, w3, b3,
           pw1, pw2, cw1, cw2, cb2, wout, bout):
    global LAST_RESULTS
    from concourse.bass_utils import run_bass_kernel_spmd

    consts = _build_consts(w0, b0, w1, b1, w2, b2, w3, b3, pw1, pw2,
                           cw1, cw2, cb2, wout, bout)
    nc = _build_program()
    _legalize_waits(nc)

    in_maps = []
    for i in range(NCORES):
        sl = slice(i * NS, (i + 1) * NS)
        m = dict(consts)
        m["feats"] = _pack_feats(np.ascontiguousarray(sm_feats[:, :, sl, :]))
        m["ppfs"] = _pack_ppfs(np.ascontiguousarray(sm_ppfs[:, :, sl, :]))
        in_maps.append(m)

    res = run_bass_kernel_spmd(nc, in_maps, list(range(NCORES)))
    LAST_RESULTS = res
    shards = [res.results[i]["out"].astype(np.float32) for i in range(NCORES)]
    return np.concatenate(shards, axis=2)


# revision 34
# speedup vs baseline: 1.2288x; 1.0696x over previous
"""Point-Transformer attention block on 8 Trainium2 NeuronCores.

Shards the points axis N across 8 cores (all ops are pointwise in N).
Per core: channels on SBUF partitions, pixels (k-major: k*TP+n) on the
free dim.  All matmuls run in bf16 (1 cycle/row vs 4 for fp32); inputs
are converted + laid out k-major on the host so every DMA is contiguous.

Fusions vs the reference:
  - x2/x3 in one matmul: stationary [w3; -w2] -> psum [x3_pre+ptf | d]
  - ptf accumulated into the same psum via [pw2; pw2]
  - pw1 block-diag stacked: two 512-px chunks per pass on 128 partitions
  - x1 / xfs / cw1 folded: h = relu(sum_k cw1_k^T d_k + (cw1s@w1)@x
                                    + cw1s@(b1-b2))
  - softmax: exp -> group-sum matmul -> reciprocal; denominator applied
    after the weighted k-sum
  - identity skip-connection via GpSimd add (no matmul)
"""

import numpy as np
import ml_dtypes

B, CIN, N, K = 4, 64, 16384, 16
MID, OUT, PT, SHARE = 64, 128, 8, 8
G = MID // SHARE          # 8 softmax groups
NCORES = 8
NS = N // NCORES          # points per core per batch (2048)
TP = 256                  # points per tile
TPK = TP * K              # pixels per tile (4096)
NT = NS // TP             # tiles per batch (8)
CH = 1024                 # pixel-chunk columns (2 psum banks)
NCH = TPK // CH           # 4 chunks per tile
BF16 = ml_dtypes.bfloat16


def _build_consts(w0, b0, w1, b1, w2, b2, w3, b3, pw1, pw2, cw1, cw2, cb2,
                  wout, bout):
    f32, bf = np.float32, BF16
    c = {}
    w0t = w0.T.astype(np.float32)                                   # [64,128]
    c["w0t2"] = np.ascontiguousarray(
        np.concatenate([w0t, w0t], axis=0), bf)                     # [128,128]
    c["w23t"] = np.ascontiguousarray(
        np.concatenate([w3, -w2], axis=0).T, bf)                    # [128,128]
    pw2t2 = np.concatenate([pw2, pw2], axis=0).T                    # [64,128]
    c["pw2d"] = np.ascontiguousarray(
        np.concatenate([pw2t2, pw2t2], axis=0), bf)                 # [128,128]
    pw1t = pw1.T.astype(np.float32)                                 # [8,64]
    pw1q = np.zeros((OUT, OUT), np.float32)
    for q in range(4):
        for s in range(2):
            pw1q[32 * q + 8 * s:32 * q + 8 * s + PT,
                 s * MID:s * MID + MID] = pw1t
    c["pw1q"] = np.ascontiguousarray(pw1q, bf)                      # [128,128]
    cw1r = cw1.reshape(G, MID, K)
    cw1s = cw1r.sum(-1)                                             # [8,64]
    c["cat"] = np.ascontiguousarray((cw1s @ w1).T, bf)              # [128,8]
    gktp = np.zeros((OUT, (K // 2) * G), np.float32)
    for kp in range(K // 2):
        gktp[0:MID, kp * G:(kp + 1) * G] = cw1r[:, :, 2 * kp].T
        gktp[MID:OUT, kp * G:(kp + 1) * G] = cw1r[:, :, 2 * kp + 1].T
    c["gktp"] = np.ascontiguousarray(gktp, bf)                      # [128,64]
    c["hb"] = np.ascontiguousarray((cw1s @ (b1 - b2))[:, None], f32)  # [8,1]
    c["cw2t"] = np.ascontiguousarray(cw2.T, bf)                     # [8,128]
    c["cb2"] = np.ascontiguousarray(cb2[:, None], f32)              # [128,1]
    bsum = np.zeros((OUT, G), np.float32)
    for g in range(G):
        bsum[g * K:(g + 1) * K, g] = 1.0
    c["bsum"] = np.ascontiguousarray(bsum, bf)                      # [128,8]
    c["grep"] = np.ascontiguousarray(bsum.T, bf)                    # [8,128]
    bksel2 = np.zeros((OUT, (K // 2) * OUT), np.float32)
    for kp in range(K // 2):
        for j in range(2):
            for m in range(MID):
                bksel2[(m % G) * K + 2 * kp + j,
                       kp * OUT + j * MID + m] = 1.0
    c["bksel2"] = np.ascontiguousarray(bksel2, bf)                  # [128,1024]
    fold2 = np.zeros((OUT, MID), np.float32)
    for j in range(2):
        for m in range(MID):
            fold2[j * MID + m, m] = 1.0
    c["fold2"] = np.ascontiguousarray(fold2, bf)                    # [128,64]
    c["woutt"] = np.ascontiguousarray(wout.T, bf)                   # [64,128]
    c["b0"] = np.ascontiguousarray(b0[:, None], f32)                # [128,1]
    b3z = np.zeros((OUT, 1), np.float32)
    b3z[0:MID, 0] = b3
    c["b3z"] = b3z                                                  # [128,1]
    c["bout"] = np.ascontiguousarray(bout[:, None], f32)            # [128,1]
    return c


CONST_SHAPES = dict(
    w0t2=[OUT, OUT], w23t=[OUT, OUT], pw2d=[OUT, OUT], pw1q=[OUT, OUT],
    cat=[OUT, G], gktp=[OUT, K * G // 2], hb=[G, 1], cw2t=[G, OUT],
    cb2=[OUT, 1], bsum=[OUT, G], grep=[G, OUT],
    bksel2=[OUT, K * OUT // 2], fold2=[OUT, MID], woutt=[MID, OUT],
    b0=[OUT, 1], b3z=[OUT, 1], bout=[OUT, 1],
)
CONST_F32 = {"hb", "cb2", "b0", "b3z", "bout"}


def _build_program():
    import concourse.bass as bass
    import concourse.tile as tile
    from concourse import mybir
    from contextlib import ExitStack

    f32 = mybir.dt.float32
    bf16 = mybir.dt.bfloat16
    AF = mybir.ActivationFunctionType
    ALU = mybir.AluOpType

    nc = bass.Bass()
    feats_d = nc.declare_dram_parameter("feats", [B, 2 * CIN, NS * K // 2],
                                        bf16, isOutput=False)
    ppfs_d = nc.declare_dram_parameter("ppfs", [B, OUT, NT * 512], bf16,
                                       isOutput=False)
    cdram = {k: nc.declare_dram_parameter(
                 k, v, f32 if k in CONST_F32 else bf16, isOutput=False)
             for k, v in CONST_SHAPES.items()}
    out_d = nc.declare_dram_parameter("out", [B, OUT, NS], bf16, isOutput=True)

    with tile.TileContext(nc) as tc, ExitStack() as ctx:
        consts = ctx.enter_context(tc.tile_pool(name="consts", bufs=1))
        ct = {k: consts.tile_from(v[:], name=k) for k, v in cdram.items()}

        io = ctx.enter_context(tc.tile_pool(name="io", bufs=3))
        sb4 = ctx.enter_context(tc.tile_pool(name="sb4", bufs=5))
        sb3 = ctx.enter_context(tc.tile_pool(name="sb3", bufs=3))
        sb = ctx.enter_context(tc.tile_pool(name="sb", bufs=2))
        pt_pool = ctx.enter_context(tc.tile_pool(name="pt", bufs=4))
        ps_pix = ctx.enter_context(tc.tile_pool(name="ps_pix", bufs=2,
                                                space="PSUM"))
        ps_wf = ctx.enter_context(tc.tile_pool(name="ps_wf", bufs=2,
                                               space="PSUM"))
        ps_pt = ctx.enter_context(tc.tile_pool(name="ps_pt", bufs=2,
                                               space="PSUM"))

        def emit_s1(b, t):
            """Pixel phase: xn / r / [x3|d] + parity repacks (DMA)."""
            pk = slice(t * TPK // 2, (t + 1) * TPK // 2)
            ph = slice(t * 512, (t + 1) * 512)

            ft = io.tile([OUT, TPK // 2], bf16, tag="ft")
            nc.sync.dma_start(ft[:], feats_d[b, :, pk])
            pf = io.tile([OUT, 512], bf16, tag="pf")
            nc.sync.dma_start(pf[:], ppfs_d[b, :, ph])

            xn = sb4.tile([OUT, TPK], bf16, tag="xn")
            x3dd = sb.tile([OUT, TPK], bf16, tag="x3dd")
            r = sb.tile([OUT, TPK // 2], bf16, tag="r")

            # r: 4-way row-tiled pw1 (concurrent in PE quadrant rows)
            for i in range(2):
                p = ps_pix.tile([OUT, CH], f32, tag="pix")
                for j in range(2):
                    q = 2 * i + j
                    nc.tensor.matmul(out=p[:, j * 512:(j + 1) * 512],
                                     lhsT=ct["pw1q"][32 * q:32 * q + 16, :],
                                     rhs=pf[32 * q:32 * q + 16, :],
                                     start=True, stop=True,
                                     tile_position=(32 * q, 0))
                nc.vector.tensor_scalar_max(
                    r[:, i * CH:(i + 1) * CH], p[:], 0.0)

            # xn = relu(w0 @ feats + b0): k-halves row-tiled concurrent
            for c in range(NCH):
                p = ps_pix.tile([OUT, CH], f32, tag="pix")
                cs = slice(c * 512, (c + 1) * 512)
                nc.tensor.matmul(out=p[:, 0:512],
                                 lhsT=ct["w0t2"][0:CIN, :], rhs=ft[0:CIN, cs],
                                 start=True, stop=True)
                nc.tensor.matmul(out=p[:, 512:1024],
                                 lhsT=ct["w0t2"][CIN:OUT, :],
                                 rhs=ft[CIN:OUT, cs], start=True, stop=True)
                xo = xn[:].rearrange("p (h n) -> p h n", h=2)[:, :,
                                                             c * 512:(c + 1) * 512]
                pv = p[:].rearrange("p (h n) -> p h n", h=2)
                nc.scalar.activation(xo, pv, AF.Relu, bias=ct["b0"][:])

            # psum = [w3@xn + pw2@r | -w2@xn + pw2@r] -> x3 (+b3), d
            for i in range(NCH):
                p = ps_pix.tile([OUT, CH], f32, tag="pix")
                cs = slice(i * CH, (i + 1) * CH)
                for j in range(2):
                    js = slice(i * CH + j * 512, i * CH + (j + 1) * 512)
                    nc.tensor.matmul(out=p[:, j * 512:(j + 1) * 512],
                                     lhsT=ct["w23t"][:],
                                     rhs=xn[:, js], start=True, stop=False)
                for j in range(2):
                    nc.tensor.matmul(
                        out=p[:, j * 512:(j + 1) * 512],
                        lhsT=ct["pw2d"][j * MID:(j + 1) * MID, :],
                        rhs=r[j * MID:(j + 1) * MID, i * 512:(i + 1) * 512],
                        start=False, stop=True)
                if i < 3:
                    nc.scalar.activation(x3dd[:, cs], p[:], AF.Identity,
                                         bias=ct["b3z"][:])
                else:
                    nc.vector.tensor_scalar_add(x3dd[:, cs], p[:],
                                                ct["b3z"][:])

            # parity repack via SBUF->SBUF DMA: top=even k, bottom=odd k
            x3p = sb3.tile([OUT, TPK // 2], bf16, tag="x3p")
            ddp = sb3.tile([OUT, TPK // 2], bf16, tag="ddp")
            v = x3dd[:].rearrange("p (k2 two n) -> p two k2 n", two=2, n=TP)
            x3pv = x3p[:].rearrange("p (k2 n) -> p k2 n", n=TP)
            ddpv = ddp[:].rearrange("p (k2 n) -> p k2 n", n=TP)
            nc.sync.dma_start(x3pv[0:MID], v[0:MID, 0])
            nc.sync.dma_start(x3pv[MID:OUT], v[0:MID, 1])
            nc.sync.dma_start(ddpv[0:MID], v[MID:OUT, 0])
            nc.sync.dma_start(ddpv[MID:OUT], v[MID:OUT, 1])
            return dict(b=b, t=t, xn=xn, x3p=x3p, ddp=ddp)

        def emit_s2(st):
            """h = relu(sum_k cw1_k^T d_k + Ca x + hb); paired contraction."""
            sm1 = ps_pt.tile([OUT, 512], f32, tag="pt")
            hps = sm1[0:G, 256:512]
            ddp = st["ddp"]
            for kp in range(K // 2):
                nc.tensor.matmul(
                    out=hps, lhsT=ct["gktp"][:, kp * G:(kp + 1) * G],
                    rhs=ddp[:, kp * TP:(kp + 1) * TP],
                    start=(kp == 0), stop=False)
            nc.tensor.matmul(out=hps, lhsT=ct["cat"][:],
                             rhs=st["xn"][:, 0:TP], start=False, stop=True)
            h = pt_pool.tile([G, TP], bf16, tag="h")
            nc.scalar.activation(h[:], hps, AF.Relu, bias=ct["hb"][:])
            st.update(sm1=sm1, h=h)

        def emit_cw2(st):
            # e = exp(cw2 @ h + cb2)                          [128, TP]
            sm1 = st["sm1"]
            wlps = sm1[:, 0:256]
            nc.tensor.matmul(out=wlps, lhsT=ct["cw2t"][:], rhs=st["h"][:],
                             start=True, stop=True)
            e = pt_pool.tile([OUT, TP], bf16, tag="e")
            nc.scalar.activation(e[:], wlps, AF.Exp, bias=ct["cb2"][:])
            st.update(e=e)

        def emit_s3a(st):
            sm1, x3p, e = st["sm1"], st["x3p"], st["e"]

            # rs = 1 / group-sum(e)   (out base partition 32-aligned)
            sps = sm1[32:32 + G, 256:512]
            nc.tensor.matmul(out=sps, lhsT=ct["bsum"][:], rhs=e[:],
                             start=True, stop=True)
            rs32 = pt_pool.tile([G, TP], f32, tag="rs32")
            nc.vector.reciprocal(rs32[:], sps)
            rs16 = pt_pool.tile([G, TP], bf16, tag="rs16")
            nc.gpsimd.tensor_copy(rs16[:], rs32[:])

            # en = softmax(e): e * (1/groupsum) broadcast     [128, TP]
            rbps = sm1[:, 0:256]  # reuse wl region after e drained
            nc.tensor.matmul(out=rbps, lhsT=ct["grep"][:], rhs=rs16[:],
                             start=True, stop=True)
            en = pt_pool.tile([OUT, TP], bf16, tag="en")
            nc.vector.tensor_mul(en[:], e[:], rbps)

            # wf pairs: psum [128,TP] = [wf_{2kp} | wf_{2kp+1}]; q2 = wf*x3p
            q2 = sb.tile([OUT, TPK // 2], bf16, tag="q2")
            for inst in range(4):
                wfp = ps_wf.tile([OUT, 512], f32, tag="wf")
                for j in range(2):
                    kp = inst * 2 + j
                    nc.tensor.matmul(
                        out=wfp[:, j * TP:(j + 1) * TP],
                        lhsT=ct["bksel2"][:, kp * OUT:(kp + 1) * OUT],
                        rhs=en[:], start=True, stop=True)
                cs = slice(inst * 512, (inst + 1) * 512)
                nc.vector.tensor_mul(q2[:, cs], wfp[:], x3p[:, cs])

            # U = sum_k q: pairwise-add tree on GpSimd (SBUF bf16)
            t1 = pt_pool.tile([OUT, CH], bf16, tag="t1")
            t2 = pt_pool.tile([OUT, 512], bf16, tag="t2")
            ueo = pt_pool.tile([OUT, TP], bf16, tag="ueo")
            with nc.allow_low_precision("8-term bf16 partial sums"):
                nc.gpsimd.tensor_add(t1[:], q2[:, 0:CH], q2[:, CH:2 * CH])
                nc.gpsimd.tensor_add(t2[:], t1[:, 0:512], t1[:, 512:CH])
                nc.gpsimd.tensor_add(ueo[:], t2[:, 0:TP], t2[:, TP:512])
            st.update(ueo=ueo)

        def emit_fold(st):
            # fold parity halves of ueo via matmul; o = relu(U)
            wop = ps_wf.tile([OUT, 512], f32, tag="wf")
            ups = wop[0:MID, 256:512]
            nc.tensor.matmul(out=ups, lhsT=ct["fold2"][:], rhs=st["ueo"][:],
                             start=True, stop=True)
            o = pt_pool.tile([MID, TP], bf16, tag="o")
            nc.scalar.activation(o[:], ups, AF.Relu)
            st.update(o=o, wop=wop)

        def emit_s3b(st):
            b, t = st["b"], st["t"]
            # out = wout @ o + bout + x                       [128, TP]
            ops_ = st["wop"][:, 0:256]
            nc.tensor.matmul(out=ops_, lhsT=ct["woutt"][:], rhs=st["o"][:],
                             start=True, stop=True)
            res = pt_pool.tile([OUT, TP], bf16, tag="res")
            nc.vector.tensor_scalar_add(res[:], ops_, ct["bout"][:])
            outt = pt_pool.tile([OUT, TP], bf16, tag="outt")
            nc.gpsimd.tensor_add(outt[:], res[:], st["xn"][:, 0:TP])
            nc.sync.dma_start(out_d[b, :, t * TP:(t + 1) * TP], outt[:])

        tiles = [(b, t) for b in range(B) for t in range(NT)] + [None] * 4
        p1 = p2 = p3 = p4 = None
        for tile_idx in tiles:
            if p4 is not None:
                emit_fold(p4)
            if p1 is not None:
                emit_s2(p1)
            if p2 is not None:
                emit_s3a(p2)
            if p1 is not None:
                emit_cw2(p1)
            cur = emit_s1(*tile_idx) if tile_idx is not None else None
            if p4 is not None:
                emit_s3b(p4)
            p4, p3, p2, p1 = p3, p2, p1, cur

    return nc


def _legalize_waits(nc):
    """This toolchain's walrus rejects >1 sync-wait per instruction; hoist
    extra waits onto same-engine event-semaphore instructions just before."""
    from concourse import mybir

    n_split = 0
    for fn in nc.m.functions:
        for bb in fn.blocks:
            insts = bb.instructions
            new_list = []
            for inst in insts:
                si = inst.sync_info
                if si is not None and si.on_wait is not None and len(si.on_wait) > 1:
                    waits = list(si.on_wait)
                    for j, w in enumerate(waits[:-1]):
                        ev = mybir.InstEventSemaphore(
                            name=f"{inst.name}-lw{j}", ins=[], outs=[])
                        ev.engine = inst.engine
                        ev.sync_info = mybir.SyncInfo(on_wait=[w], on_update=[])
                        new_list.append(ev)
                        n_split += 1
                    inst.sync_info = mybir.SyncInfo(
                        on_wait=[waits[-1]], on_update=list(si.on_update))
                new_list.append(inst)
            if len(new_list) != len(insts):
                insts[:] = new_list
    return n_split


def _pack_feats(x):
    # [B, CIN, NS, K] -> per-tile k-major cols, k-halves stacked on
    # partitions: out[b, h*64+c, t, k8*TP+n] = x[b, c, t*TP+n, h*8+k8]
    a = x.reshape(B, CIN, NT, TP, 2, K // 2).transpose(0, 4, 1, 2, 5, 3)
    return np.ascontiguousarray(a, BF16).reshape(B, 2 * CIN, NS * K // 2)


def _pack_ppfs(x):
    # [B, PT, NS, K] -> per-tile [128, 512] for 4-way row-tiled pw1:
    # partition 32q+8s+c holds pixel cols q*1024+s*512+j (j = k1*TP+n)
    a = x.reshape(B, PT, NT, TP, K).transpose(0, 1, 2, 4, 3)  # [B,8,NT,K,TP]
    out = np.zeros((B, OUT, NT, 512), BF16)
    for k in range(K):
        q, s, k1 = k // 4, (k // 2) % 2, k % 2
        out[:, 32 * q + 8 * s:32 * q + 8 * s + PT, :,
            k1 * TP:(k1 + 1) * TP] = a[:, :, :, k, :]
    return np.ascontiguousarray(out).reshape(B, OUT, NT * 512)


LAST_RESULTS = None


def kernel(sm_feats, sm_ppfs, w0, b0, w1, b1, w2, b2, w3, b3,
           pw1, pw2, cw1, cw2, cb2, wout, bout):
    global LAST_RESULTS
    from concourse.bass_utils import run_bass_kernel_spmd

    consts = _build_consts(w0, b0, w1, b1, w2, b2, w3, b3, pw1, pw2,
                           cw1, cw2, cb2, wout, bout)
    nc = _build_program()
    _legalize_waits(nc)

    in_maps = []
    for i in range(NCORES):
        sl = slice(i * NS, (i + 1) * NS)
        m = dict(consts)
        m["feats"] = _pack_feats(np.ascontiguousarray(sm_feats[:, :, sl, :]))
        m["ppfs"] = _pack_ppfs(np.ascontiguousarray(sm_ppfs[:, :, sl, :]))
        in_maps.append(m)

    res = run_bass_kernel_spmd(nc, in_maps, list(range(NCORES)))
    LAST_RESULTS = res
    shards = [res.results[i]["out"].astype(np.float32) for i in range(NCORES)]
    return np.concatenate(shards, axis=2)


# revision 38
# speedup vs baseline: 1.2344x; 1.0046x over previous
"""Point-Transformer attention block on 8 Trainium2 NeuronCores.

Shards the points axis N across 8 cores (all ops are pointwise in N).
Per core: channels on SBUF partitions, pixels (k-major: k*TP+n) on the
free dim.  All matmuls run in bf16 (1 cycle/row vs 4 for fp32); inputs
are converted + laid out k-major on the host so every DMA is contiguous.

Fusions vs the reference:
  - x2/x3 in one matmul: stationary [w3; -w2] -> psum [x3_pre+ptf | d]
  - ptf accumulated into the same psum via [pw2; pw2]
  - pw1 block-diag stacked: two 512-px chunks per pass on 128 partitions
  - x1 / xfs / cw1 folded: h = relu(sum_k cw1_k^T d_k + (cw1s@w1)@x
                                    + cw1s@(b1-b2))
  - softmax: exp -> group-sum matmul -> reciprocal; denominator applied
    after the weighted k-sum
  - identity skip-connection via GpSimd add (no matmul)
"""

import numpy as np
import ml_dtypes

B, CIN, N, K = 4, 64, 16384, 16
MID, OUT, PT, SHARE = 64, 128, 8, 8
G = MID // SHARE          # 8 softmax groups
NCORES = 8
NS = N // NCORES          # points per core per batch (2048)
TP = 256                  # points per tile
TPK = TP * K              # pixels per tile (4096)
NT = NS // TP             # tiles per batch (8)
CH = 1024                 # pixel-chunk columns (2 psum banks)
NCH = TPK // CH           # 4 chunks per tile
BF16 = ml_dtypes.bfloat16


def _build_consts(w0, b0, w1, b1, w2, b2, w3, b3, pw1, pw2, cw1, cw2, cb2,
                  wout, bout):
    f32, bf = np.float32, BF16
    c = {}
    w0t = w0.T.astype(np.float32)                                   # [64,128]
    c["w0t2"] = np.ascontiguousarray(
        np.concatenate([w0t, w0t], axis=0), bf)                     # [128,128]
    c["w23t"] = np.ascontiguousarray(
        np.concatenate([w3, -w2], axis=0).T, bf)                    # [128,128]
    pw2t2 = np.concatenate([pw2, pw2], axis=0).T                    # [64,128]
    c["pw2d"] = np.ascontiguousarray(
        np.concatenate([pw2t2, pw2t2], axis=0), bf)                 # [128,128]
    pw1t = pw1.T.astype(np.float32)                                 # [8,64]
    pw1q = np.zeros((OUT, OUT), np.float32)
    for q in range(4):
        for s in range(2):
            pw1q[32 * q + 8 * s:32 * q + 8 * s + PT,
                 s * MID:s * MID + MID] = pw1t
    c["pw1q"] = np.ascontiguousarray(pw1q, bf)                      # [128,128]
    cw1r = cw1.reshape(G, MID, K)
    cw1s = cw1r.sum(-1)                                             # [8,64]
    c["cat"] = np.ascontiguousarray((cw1s @ w1).T, bf)              # [128,8]
    gktp = np.zeros((OUT, (K // 2) * G), np.float32)
    for kp in range(K // 2):
        gktp[0:MID, kp * G:(kp + 1) * G] = cw1r[:, :, 2 * kp].T
        gktp[MID:OUT, kp * G:(kp + 1) * G] = cw1r[:, :, 2 * kp + 1].T
    c["gktp"] = np.ascontiguousarray(gktp, bf)                      # [128,64]
    c["hb"] = np.ascontiguousarray((cw1s @ (b1 - b2))[:, None], f32)  # [8,1]
    c["cw2t"] = np.ascontiguousarray(cw2.T, bf)                     # [8,128]
    c["cb2"] = np.ascontiguousarray(cb2[:, None], f32)              # [128,1]
    bsum = np.zeros((OUT, G), np.float32)
    for g in range(G):
        bsum[g * K:(g + 1) * K, g] = 1.0
    c["bsum"] = np.ascontiguousarray(bsum, bf)                      # [128,8]
    c["grep"] = np.ascontiguousarray(bsum.T, bf)                    # [8,128]
    bksel2 = np.zeros((OUT, (K // 2) * OUT), np.float32)
    for kp in range(K // 2):
        for j in range(2):
            for m in range(MID):
                bksel2[(m % G) * K + 2 * kp + j,
                       kp * OUT + j * MID + m] = 1.0
    c["bksel2"] = np.ascontiguousarray(bksel2, bf)                  # [128,1024]
    fold2 = np.zeros((OUT, MID), np.float32)
    for j in range(2):
        for m in range(MID):
            fold2[j * MID + m, m] = 1.0
    c["fold2"] = np.ascontiguousarray(fold2, bf)                    # [128,64]
    c["woutt"] = np.ascontiguousarray(wout.T, bf)                   # [64,128]
    c["b0"] = np.ascontiguousarray(b0[:, None], f32)                # [128,1]
    b3z = np.zeros((OUT, 1), np.float32)
    b3z[0:MID, 0] = b3
    c["b3z"] = b3z                                                  # [128,1]
    c["bout"] = np.ascontiguousarray(bout[:, None], f32)            # [128,1]
    return c


CONST_SHAPES = dict(
    w0t2=[OUT, OUT], w23t=[OUT, OUT], pw2d=[OUT, OUT], pw1q=[OUT, OUT],
    cat=[OUT, G], gktp=[OUT, K * G // 2], hb=[G, 1], cw2t=[G, OUT],
    cb2=[OUT, 1], bsum=[OUT, G], grep=[G, OUT],
    bksel2=[OUT, K * OUT // 2], fold2=[OUT, MID], woutt=[MID, OUT],
    b0=[OUT, 1], b3z=[OUT, 1], bout=[OUT, 1],
)
CONST_F32 = {"hb", "cb2", "b0", "b3z", "bout"}


def _build_program():
    import concourse.bass as bass
    import concourse.tile as tile
    from concourse import mybir
    from contextlib import ExitStack

    f32 = mybir.dt.float32
    bf16 = mybir.dt.bfloat16
    AF = mybir.ActivationFunctionType
    ALU = mybir.AluOpType

    nc = bass.Bass()
    feats_d = nc.declare_dram_parameter("feats", [B, 2 * CIN, NS * K // 2],
                                        bf16, isOutput=False)
    ppfs_d = nc.declare_dram_parameter("ppfs", [B, OUT, NT * 512], bf16,
                                       isOutput=False)
    cdram = {k: nc.declare_dram_parameter(
                 k, v, f32 if k in CONST_F32 else bf16, isOutput=False)
             for k, v in CONST_SHAPES.items()}
    out_d = nc.declare_dram_parameter("out", [B, OUT, NS], bf16, isOutput=True)

    with tile.TileContext(nc) as tc, ExitStack() as ctx:
        consts = ctx.enter_context(tc.tile_pool(name="consts", bufs=1))
        ct = {k: consts.tile_from(v[:], name=k) for k, v in cdram.items()}

        io = ctx.enter_context(tc.tile_pool(name="io", bufs=3))
        sb4 = ctx.enter_context(tc.tile_pool(name="sb4", bufs=5))
        sb3 = ctx.enter_context(tc.tile_pool(name="sb3", bufs=3))
        sb = ctx.enter_context(tc.tile_pool(name="sb", bufs=2))
        pt_pool = ctx.enter_context(tc.tile_pool(name="pt", bufs=4))
        ps_pix = ctx.enter_context(tc.tile_pool(name="ps_pix", bufs=2,
                                                space="PSUM"))
        ps_wf = ctx.enter_context(tc.tile_pool(name="ps_wf", bufs=2,
                                               space="PSUM"))
        ps_pt = ctx.enter_context(tc.tile_pool(name="ps_pt", bufs=2,
                                               space="PSUM"))

        def emit_s1(b, t):
            """Pixel phase: xn / r / [x3|d] + parity repacks (DMA)."""
            pk = slice(t * TPK // 2, (t + 1) * TPK // 2)
            ph = slice(t * 512, (t + 1) * 512)

            ft = io.tile([OUT, TPK // 2], bf16, tag="ft")
            nc.sync.dma_start(ft[:], feats_d[b, :, pk])
            pf = io.tile([OUT, 512], bf16, tag="pf")
            nc.sync.dma_start(pf[:], ppfs_d[b, :, ph])

            xn = sb4.tile([OUT, TPK], bf16, tag="xn")
            x3dd = sb.tile([OUT, TPK], bf16, tag="x3dd")
            r = sb.tile([OUT, TPK // 2], bf16, tag="r")

            # r: 4-way row-tiled pw1 (concurrent in PE quadrant rows)
            for i in range(2):
                p = ps_pix.tile([OUT, CH], f32, tag="pix")
                for j in range(2):
                    q = 2 * i + j
                    nc.tensor.matmul(out=p[:, j * 512:(j + 1) * 512],
                                     lhsT=ct["pw1q"][32 * q:32 * q + 16, :],
                                     rhs=pf[32 * q:32 * q + 16, :],
                                     start=True, stop=True,
                                     tile_position=(32 * q, 0))
                nc.vector.tensor_scalar_max(
                    r[:, i * CH:(i + 1) * CH], p[:], 0.0)

            # xn = relu(w0 @ feats + b0): k-halves row-tiled concurrent
            for c in range(NCH):
                p = ps_pix.tile([OUT, CH], f32, tag="pix")
                cs = slice(c * 512, (c + 1) * 512)
                nc.tensor.matmul(out=p[:, 0:512],
                                 lhsT=ct["w0t2"][0:CIN, :], rhs=ft[0:CIN, cs],
                                 start=True, stop=True)
                nc.tensor.matmul(out=p[:, 512:1024],
                                 lhsT=ct["w0t2"][CIN:OUT, :],
                                 rhs=ft[CIN:OUT, cs], start=True, stop=True)
                xo = xn[:].rearrange("p (h n) -> p h n", h=2)[:, :,
                                                             c * 512:(c + 1) * 512]
                pv = p[:].rearrange("p (h n) -> p h n", h=2)
                nc.scalar.activation(xo, pv, AF.Relu, bias=ct["b0"][:])

            return dict(b=b, t=t, xn=xn, r=r, x3dd=x3dd)

        def emit_s1b(st):
            xn, r, x3dd = st["xn"], st["r"], st["x3dd"]
            # psum = [w3@xn + pw2@r | -w2@xn + pw2@r] -> x3 (+b3), d
            for i in range(NCH):
                p = ps_pix.tile([OUT, CH], f32, tag="pix")
                cs = slice(i * CH, (i + 1) * CH)
                for j in range(2):
                    js = slice(i * CH + j * 512, i * CH + (j + 1) * 512)
                    nc.tensor.matmul(out=p[:, j * 512:(j + 1) * 512],
                                     lhsT=ct["w23t"][:],
                                     rhs=xn[:, js], start=True, stop=False)
                for j in range(2):
                    nc.tensor.matmul(
                        out=p[:, j * 512:(j + 1) * 512],
                        lhsT=ct["pw2d"][j * MID:(j + 1) * MID, :],
                        rhs=r[j * MID:(j + 1) * MID, i * 512:(i + 1) * 512],
                        start=False, stop=True)
                if i < 3:
                    nc.scalar.activation(x3dd[:, cs], p[:], AF.Identity,
                                         bias=ct["b3z"][:])
                else:
                    nc.vector.tensor_scalar_add(x3dd[:, cs], p[:],
                                                ct["b3z"][:])

            # parity repack via SBUF->SBUF DMA: top=even k, bottom=odd k
            x3p = sb3.tile([OUT, TPK // 2], bf16, tag="x3p")
            ddp = sb3.tile([OUT, TPK // 2], bf16, tag="ddp")
            v = x3dd[:].rearrange("p (k2 two n) -> p two k2 n", two=2, n=TP)
            x3pv = x3p[:].rearrange("p (k2 n) -> p k2 n", n=TP)
            ddpv = ddp[:].rearrange("p (k2 n) -> p k2 n", n=TP)
            nc.sync.dma_start(x3pv[0:MID], v[0:MID, 0])
            nc.sync.dma_start(x3pv[MID:OUT], v[0:MID, 1])
            nc.sync.dma_start(ddpv[0:MID], v[MID:OUT, 0])
            nc.sync.dma_start(ddpv[MID:OUT], v[MID:OUT, 1])
            st.update(x3p=x3p, ddp=ddp)

        def emit_s2(st):
            """h = relu(sum_k cw1_k^T d_k + Ca x + hb); paired contraction."""
            sm1 = ps_pt.tile([OUT, 512], f32, tag="pt")
            hps = sm1[0:G, 256:512]
            ddp = st["ddp"]
            for kp in range(K // 2):
                nc.tensor.matmul(
                    out=hps, lhsT=ct["gktp"][:, kp * G:(kp + 1) * G],
                    rhs=ddp[:, kp * TP:(kp + 1) * TP],
                    start=(kp == 0), stop=False)
            nc.tensor.matmul(out=hps, lhsT=ct["cat"][:],
                             rhs=st["xn"][:, 0:TP], start=False, stop=True)
            h = pt_pool.tile([G, TP], bf16, tag="h")
            nc.scalar.activation(h[:], hps, AF.Relu, bias=ct["hb"][:])
            st.update(sm1=sm1, h=h)

        def emit_cw2(st):
            # e = exp(cw2 @ h + cb2)                          [128, TP]
            sm1 = st["sm1"]
            wlps = sm1[:, 0:256]
            nc.tensor.matmul(out=wlps, lhsT=ct["cw2t"][:], rhs=st["h"][:],
                             start=True, stop=True)
            e = pt_pool.tile([OUT, TP], bf16, tag="e")
            nc.scalar.activation(e[:], wlps, AF.Exp, bias=ct["cb2"][:])
            st.update(e=e)

        def emit_s3a(st):
            sm1, x3p, e = st["sm1"], st["x3p"], st["e"]

            # rs = 1 / group-sum(e)   (out base partition 32-aligned)
            sps = sm1[32:32 + G, 256:512]
            nc.tensor.matmul(out=sps, lhsT=ct["bsum"][:], rhs=e[:],
                             start=True, stop=True)
            rs32 = pt_pool.tile([G, TP], f32, tag="rs32")
            nc.vector.reciprocal(rs32[:], sps)
            rs16 = pt_pool.tile([G, TP], bf16, tag="rs16")
            nc.gpsimd.tensor_copy(rs16[:], rs32[:])

            # en = softmax(e): e * (1/groupsum) broadcast     [128, TP]
            rbps = sm1[:, 0:256]  # reuse wl region after e drained
            nc.tensor.matmul(out=rbps, lhsT=ct["grep"][:], rhs=rs16[:],
                             start=True, stop=True)
            en = pt_pool.tile([OUT, TP], bf16, tag="en")
            nc.vector.tensor_mul(en[:], e[:], rbps)

            # wf pairs: psum [128,TP] = [wf_{2kp} | wf_{2kp+1}]; q2 = wf*x3p
            q2 = sb.tile([OUT, TPK // 2], bf16, tag="q2")
            for inst in range(2):
                wfp = ps_wf.tile([OUT, 512], f32, tag="wf")
                for j in range(2):
                    kp = inst * 2 + j
                    nc.tensor.matmul(
                        out=wfp[:, j * TP:(j + 1) * TP],
                        lhsT=ct["bksel2"][:, kp * OUT:(kp + 1) * OUT],
                        rhs=en[:], start=True, stop=True)
                cs = slice(inst * 512, (inst + 1) * 512)
                nc.vector.tensor_mul(q2[:, cs], wfp[:], x3p[:, cs])
            st.update(en=en, q2=q2)

        def emit_s3a2(st):
            x3p, en, q2 = st["x3p"], st["en"], st["q2"]
            for inst in range(2, 4):
                wfp = ps_wf.tile([OUT, 512], f32, tag="wf")
                for j in range(2):
                    kp = inst * 2 + j
                    nc.tensor.matmul(
                        out=wfp[:, j * TP:(j + 1) * TP],
                        lhsT=ct["bksel2"][:, kp * OUT:(kp + 1) * OUT],
                        rhs=en[:], start=True, stop=True)
                cs = slice(inst * 512, (inst + 1) * 512)
                nc.vector.tensor_mul(q2[:, cs], wfp[:], x3p[:, cs])

            # U = sum_k q: pairwise-add tree on GpSimd (SBUF bf16)
            t1 = pt_pool.tile([OUT, CH], bf16, tag="t1")
            t2 = pt_pool.tile([OUT, 512], bf16, tag="t2")
            ueo = pt_pool.tile([OUT, TP], bf16, tag="ueo")
            with nc.allow_low_precision("8-term bf16 partial sums"):
                nc.gpsimd.tensor_add(t1[:], q2[:, 0:CH], q2[:, CH:2 * CH])
                nc.gpsimd.tensor_add(t2[:], t1[:, 0:512], t1[:, 512:CH])
                nc.gpsimd.tensor_add(ueo[:], t2[:, 0:TP], t2[:, TP:512])
            st.update(ueo=ueo)

        def emit_fold(st):
            # fold parity halves of ueo via matmul; o = relu(U)
            wop = ps_wf.tile([OUT, 512], f32, tag="wf")
            ups = wop[0:MID, 256:512]
            nc.tensor.matmul(out=ups, lhsT=ct["fold2"][:], rhs=st["ueo"][:],
                             start=True, stop=True)
            o = pt_pool.tile([MID, TP], bf16, tag="o")
            nc.scalar.activation(o[:], ups, AF.Relu)
            st.update(o=o, wop=wop)

        def emit_s3b(st):
            b, t = st["b"], st["t"]
            # out = wout @ o + bout + x                       [128, TP]
            ops_ = st["wop"][:, 0:256]
            nc.tensor.matmul(out=ops_, lhsT=ct["woutt"][:], rhs=st["o"][:],
                             start=True, stop=True)
            res = pt_pool.tile([OUT, TP], bf16, tag="res")
            nc.vector.tensor_scalar_add(res[:], ops_, ct["bout"][:])
            outt = pt_pool.tile([OUT, TP], bf16, tag="outt")
            nc.gpsimd.tensor_add(outt[:], res[:], st["xn"][:, 0:TP])
            nc.sync.dma_start(out_d[b, :, t * TP:(t + 1) * TP], outt[:])

        tiles = [(b, t) for b in range(B) for t in range(NT)] + [None] * 4
        p1 = p2 = p3 = p4 = None
        for tile_idx in tiles:
            if p4 is not None:
                emit_fold(p4)
            if p1 is not None:
                emit_s2(p1)
            if p2 is not None:
                emit_s3a(p2)
            if p1 is not None:
                emit_cw2(p1)
            cur = emit_s1(*tile_idx) if tile_idx is not None else None
            if p2 is not None:
                emit_s3a2(p2)
            if cur is not None:
                emit_s1b(cur)
            if p4 is not None:
                emit_s3b(p4)
            p4, p3, p2, p1 = p3, p2, p1, cur

    return nc


def _legalize_waits(nc):
    """This toolchain's walrus rejects >1 sync-wait per instruction; hoist
    extra waits onto same-engine event-semaphore instructions just before."""
    from concourse import mybir

    n_split = 0
    for fn in nc.m.functions:
        for bb in fn.blocks:
            insts = bb.instructions
            new_list = []
            for inst in insts:
                si = inst.sync_info
                if si is not None and si.on_wait is not None and len(si.on_wait) > 1:
                    waits = list(si.on_wait)
                    for j, w in enumerate(waits[:-1]):
                        ev = mybir.InstEventSemaphore(
                            name=f"{inst.name}-lw{j}", ins=[], outs=[])
                        ev.engine = inst.engine
                        ev.sync_info = mybir.SyncInfo(on_wait=[w], on_update=[])
                        new_list.append(ev)
                        n_split += 1
                    inst.sync_info = mybir.SyncInfo(
                        on_wait=[waits[-1]], on_update=list(si.on_update))
                new_list.append(inst)
            if len(new_list) != len(insts):
                insts[:] = new_list
    return n_split


def _pack_feats(x):
    # [B, CIN, NS, K] -> per-tile k-major cols, k-halves stacked on
    # partitions: out[b, h*64+c, t, k8*TP+n] = x[b, c, t*TP+n, h*8+k8]
    a = x.reshape(B, CIN, NT, TP, 2, K // 2).transpose(0, 4, 1, 2, 5, 3)
    return np.ascontiguousarray(a, BF16).reshape(B, 2 * CIN, NS * K // 2)


def _pack_ppfs(x):
    # [B, PT, NS, K] -> per-tile [128, 512] for 4-way row-tiled pw1:
    # partition 32q+8s+c holds pixel cols q*1024+s*512+j (j = k1*TP+n)
    a = x.reshape(B, PT, NT, TP, K).transpose(0, 1, 2, 4, 3)  # [B,8,NT,K,TP]
    out = np.zeros((B, OUT, NT, 512), BF16)
    for k in range(K):
        q, s, k1 = k // 4, (k // 2) % 2, k % 2
        out[:, 32 * q + 8 * s:32 * q + 8 * s + PT, :,
            k1 * TP:(k1 + 1) * TP] = a[:, :, :, k, :]
    return np.ascontiguousarray(out).reshape(B, OUT, NT * 512)


LAST_RESULTS = None


def kernel(sm_feats, sm_ppfs, w0, b0, w1, b1, w2, b2, w3, b3,
           pw1, pw2, cw1, cw2, cb2, wout, bout):
    global LAST_RESULTS
    from concourse.bass_utils import run_bass_kernel_spmd

    consts = _build_consts(w0, b0, w1, b1, w2, b2, w3, b3, pw1, pw2,
                           cw1, cw2, cb2, wout, bout)
    nc = _build_program()
    _legalize_waits(nc)

    in_maps = []
    for i in range(NCORES):
        sl = slice(i * NS, (i + 1) * NS)
        m = dict(consts)
        m["feats"] = _pack_feats(np.ascontiguousarray(sm_feats[:, :, sl, :]))
        m["ppfs"] = _pack_ppfs(np.ascontiguousarray(sm_ppfs[:, :, sl, :]))
        in_maps.append(m)

    res = run_bass_kernel_spmd(nc, in_maps, list(range(NCORES)))
    LAST_RESULTS = res
    shards = [res.results[i]["out"].astype(np.float32) for i in range(NCORES)]
    return np.concatenate(shards, axis=2)
